# revision 1
# baseline (speedup 1.0000x reference)
"""Trainium2 Bass kernel for a 3-layer difflogic network (nn_Net_48610439856713).

Math: each layer o computes softmax(w[o])·ops16(a, b) with a = h[:, ia[o]],
b = h[:, ib[o]].  The 16 relaxed logic gates are all affine in {1, a, b, ab},
so the layer reduces to  h' = C0 + C1·a + C2·b + C3·a·b  with 4 per-neuron
coefficients derived on-device from softmax(w).

Sharding: 2 batch groups x 4 neuron shards over the 8 cores.  Core c handles
batch rows [(c//4)*256, ...) and neuron shard c%4 of every layer.  Activations
are bf16 in a transposed packed layout h^T[neuron, batch]; each layer's shard
outputs are exchanged with a 4-rank AllGather so every core holds the full
previous layer as its gather source.  Gathers use the SWDGE dma_gather
primitive (cost ~8ns/index of Q7 descriptor generation, the kernel's
bottleneck — which is why indices per core are minimized via neuron sharding).

Host-side bookkeeping is integer/layout only: slot permutations, index
relabeling through the packed layout, int16 index wrapping, weight-row
packing.  All float arithmetic (softmax, combine, sums) runs on device.
"""

import os
import numpy as np

P = 128
B = 512
BG = 2                  # batch groups
SH = 4                  # neuron shards
BC = B // BG            # 256 batch per core
IN = 193
NGROUP = 3
TAU = 100.0
N_CORES = 8

# layers 1/2: 16000 real neurons -> 4096 slots/shard (96 pads each)
NJ12 = 32               # j-columns per shard
REAL12 = 4000           # real neurons per shard
NS12 = NJ12 * P         # 4096 slots per shard
# layer 3: 15999 real -> 33 j-cols/shard; group g owns local j in [11g, 11g+11)
NJ3 = 33
JPG = 11                # j-cols per group per shard
NS3 = NJ3 * P           # 4224 slots per shard
SPG = 15999 // NGROUP   # 5333 real slots per group

_CACHE = {}


def _build_nc():
    import concourse.bacc as bacc
    import concourse.tile as tile
    import concourse.mybir as mybir

    f32 = mybir.dt.float32
    bf16 = mybir.dt.bfloat16
    i16 = mybir.dt.int16
    Alu = mybir.AluOpType
    Act = mybir.ActivationFunctionType
    Ax = mybir.AxisListType

    nc = bacc.Bacc("TRN2", target_bir_lowering=False, debug=False, num_devices=N_CORES)

    # ---- I/O ----
    xT = nc.dram_tensor("xT", [IN, BC], f32, kind="ExternalInput")
    wps = [
        nc.dram_tensor("w1p", [P, NJ12 * 16], f32, kind="ExternalInput"),
        nc.dram_tensor("w2p", [P, NJ12 * 16], f32, kind="ExternalInput"),
        nc.dram_tensor("w3p", [P, NJ3 * 16], f32, kind="ExternalInput"),
    ]
    idxs = []
    for l, ns in ((1, NS12), (2, NS12), (3, NS3)):
        # combined a+b index stream, chunk-interleaved: [a-chunk0 b-chunk0 ...]
        idxs.append(
            nc.dram_tensor(f"i{l}", [P, 2 * ns // 16], i16, kind="ExternalInput")
        )
    out_d = nc.dram_tensor("out", [1, NGROUP * BC], f32, kind="ExternalOutput")

    # collective buffers (h exchange, NCH j-chunks pipelined) and partial-sum
    # exchange.  g layout is chunk-major: row r = k*SH*P + s*P + p, unit
    # r*JCH + (j % JCH)  with JCH = NJ12//NCH j-cols per chunk.
    NCH_ = NCH
    JCH_ = JCH
    cins = [
        [
            nc.dram_tensor(f"cin{l}_{k}", [P, JCH * BC], bf16, kind="Internal")
            for k in range(NCH)
        ]
        for l in (1, 2)
    ]
    gs_ = [
        nc.dram_tensor("g1", [NCH * SH * P, JCH * BC], bf16, kind="Internal"),
        nc.dram_tensor("g2", [NCH * SH * P, JCH * BC], bf16, kind="Internal"),
    ]
    # warm-up collective: absorbs first-collective firmware latency while the
    # layer-1 gathers run.  Output is an (ignored) ExternalOutput so DCE keeps it.
    win = nc.dram_tensor("win", [P, 16], f32, kind="Internal")
    warm = nc.dram_tensor("warm", [SH * P, 16], f32, kind="Internal")
    pin = nc.dram_tensor("pin", [1, NGROUP * BC], f32, kind="Internal")
    pall = nc.dram_tensor("pall", [SH, NGROUP * BC], f32, kind="Internal")

    shard_groups = [[0, 1, 2, 3], [4, 5, 6, 7]]

    with tile.TileContext(nc) as tc:
        with (
            tc.tile_pool(name="big", bufs=1) as big,
            tc.tile_pool(name="prep", bufs=2) as prep,
            tc.tile_pool(name="small", bufs=2) as small,
            tc.tile_pool(name="psum", bufs=1, space="PSUM") as psum,
        ):
            layers = [
                (NJ12, NS12, f32, xT[:], idxs[0], wps[0], cins[0], gs_[0]),
                (
                    NJ12, NS12, bf16,
                    gs_[0][:].rearrange("r (j b) -> (r j) b", b=BC),
                    idxs[1], wps[1], cins[1], gs_[1],
                ),
                (
                    NJ3, NS3, bf16,
                    gs_[1][:].rearrange("r (j b) -> (r j) b", b=BC),
                    idxs[2], wps[2], None, None,
                ),
            ]  # cin entries are per-chunk lists for layers 1-2

            wsb = prep.tile([P, 16], f32, tag="wsb")
            nc.vector.memset(wsb[:], 0.0)
            nc.sync.dma_start(win[:], wsb[:])
            nc.gpsimd.collective_compute(
                "AllGather", Alu.bypass, replica_groups=shard_groups,
                ins=[win[:]], outs=[warm[:]],
            )

            h_final = None
            for li, (NJ, NS, gdt, src, iad, wp, cin, gout) in enumerate(layers):
                last = li == 2
                # ---- coefficient prep: C0..C3 [P, NJ] f32 ----
                wt = prep.tile([P, NJ * 16], f32, tag="wt")
                nc.sync.dma_start(wt[:], wp[:])
                e = prep.tile([P, NJ * 16], f32, tag="e")
                nc.scalar.activation(e[:], wt[:], Act.Exp)
                e3 = e[:].rearrange("p (j g) -> p j g", g=16)
                e4 = e[:].rearrange("p (j h q) -> p j h q", h=4, q=4)

                ssum = small.tile([P, NJ], f32, tag="ssum")
                nc.vector.reduce_sum(ssum[:], e3, axis=Ax.X)
                r = small.tile([P, NJ], f32, tag="r")
                nc.vector.reciprocal(r[:], ssum[:])

                c0 = small.tile([P, NJ], f32, tag="c0")
                c1 = small.tile([P, NJ], f32, tag="c1")
                c2 = small.tile([P, NJ], f32, tag="c2")
                c3 = small.tile([P, NJ], f32, tag="c3")

                nc.vector.reduce_sum(c0[:], e4[:, :, 2:4, :], axis=Ax.XY)
                t1 = small.tile([P, NJ], f32, tag="t1")
                t2 = small.tile([P, NJ], f32, tag="t2")
                nc.vector.reduce_sum(t1[:], e4[:, :, 0:2, 2:4], axis=Ax.XY)
                nc.vector.reduce_sum(t2[:], e4[:, :, 2:4, 0:2], axis=Ax.XY)
                nc.vector.tensor_sub(c1[:], t1[:], t2[:])
                t3 = small.tile([P, NJ], f32, tag="t3")
                t4 = small.tile([P, NJ], f32, tag="t4")
                nc.vector.reduce_sum(t3[:], e4[:, :, 1, :], axis=Ax.X)
                nc.vector.reduce_sum(t4[:], e4[:, :, 2, :], axis=Ax.X)
                nc.vector.tensor_sub(c2[:], t3[:], t4[:])
                f = small.tile([P, NJ, 7], f32, tag="f")
                nc.vector.tensor_sub(f[:], e3[:, :, 1:8], e3[:, :, 14:7:-1])
                u1 = small.tile([P, NJ], f32, tag="u1")
                u2 = small.tile([P, NJ], f32, tag="u2")
                nc.vector.tensor_sub(u1[:], f[:, :, 0], f[:, :, 1])
                nc.vector.tensor_add(u2[:], f[:, :, 3], f[:, :, 6])
                nc.vector.tensor_sub(u1[:], u1[:], u2[:])
                nc.vector.scalar_tensor_tensor(
                    c3[:], f[:, :, 5], -2.0, u1[:], op0=Alu.mult, op1=Alu.add
                )
                for ck in (c0, c1, c2, c3):
                    nc.vector.tensor_mul(ck[:], ck[:], r[:])

                # ---- idx load ----
                iab = prep.tile([P, 2 * NS // 16], i16, tag="iab")
                nc.sync.dma_start(iab[:], iad[:])

                # ---- chunked gathers + combine ----
                h = big.tile([P, NJ * BC], bf16, tag="h")
                h3 = h[:].rearrange("p (j b) -> p j b", b=BC)
                if last:
                    # group-aligned chunks so GroupSum reduces fire per chunk
                    chunks = [(0, 11), (11, 22), (22, NJ)]
                else:
                    chunks = [(k * JCH, (k + 1) * JCH) for k in range(NCH)]
                for ci, (j0, j1) in enumerate(chunks):
                    cw = j1 - j0
                    ab = big.tile([P, 2 * cw, BC], gdt, tag=f"ab{ci}")
                    nsc = 2 * cw * P
                    nc.gpsimd.dma_gather(
                        ab[:], src, iab[:, 2 * j0 * 8 : 2 * j1 * 8], nsc, nsc, BC,
                        single_packet=False,
                    )
                    tmp = big.tile([P, cw, BC], gdt, tag=f"t{ci}")
                    for j in range(j0, j1):
                        jl = j - j0
                        aj = ab[:, jl]
                        bj = ab[:, cw + jl]
                        # tmp = (a*C3)*b ; tmp = (a*C1)+tmp ; tmp = (b*C2)+tmp
                        nc.vector.scalar_tensor_tensor(
                            tmp[:, jl], aj, c3[:, j : j + 1], bj,
                            op0=Alu.mult, op1=Alu.mult,
                        )
                        nc.vector.scalar_tensor_tensor(
                            tmp[:, jl], aj, c1[:, j : j + 1], tmp[:, jl],
                            op0=Alu.mult, op1=Alu.add,
                        )
                        nc.vector.scalar_tensor_tensor(
                            tmp[:, jl], bj, c2[:, j : j + 1], tmp[:, jl],
                            op0=Alu.mult, op1=Alu.add,
                        )
                        # h = tmp + C0 on the Scalar engine
                        nc.scalar.activation(
                            h3[:, j], tmp[:, jl], Act.Identity,
                            bias=c0[:, j : j + 1], scale=1.0,
                        )

                    if not last:
                        # ship this chunk as soon as it's combined
                        nc.sync.dma_start(
                            cin[ci][:], h[:, j0 * BC : j1 * BC]
                        )
                        nc.gpsimd.collective_compute(
                            "AllGather", Alu.bypass, replica_groups=shard_groups,
                            ins=[cin[ci][:]],
                            outs=[gout[ci * SH * P : (ci + 1) * SH * P, :]],
                        )
                if last:
                    h_final = h

            # ---- GroupSum: per-shard partials, then cross-shard AllGather+sum ----
            gs = prep.tile([P, NGROUP * BC], f32, tag="gs")
            for g in range(NGROUP):
                sl = h_final[:, g * JPG * BC : (g + 1) * JPG * BC].rearrange(
                    "p (j b) -> p b j", b=BC
                )
                nc.vector.reduce_sum(gs[:, g * BC : (g + 1) * BC], sl, axis=Ax.X)
            ones = prep.tile([P, 1], f32, tag="ones")
            nc.vector.memset(ones[:], 1.0)
            psc = prep.tile([1, NGROUP * BC], f32, tag="psc")
            HW = NGROUP * BC // 2
            for k in range(2):
                ps = psum.tile([1, HW], f32, tag=f"ps{k}")
                nc.tensor.matmul(
                    ps[:], ones[:], gs[:, k * HW : (k + 1) * HW],
                    start=True, stop=True,
                )
                nc.scalar.copy(psc[:, k * HW : (k + 1) * HW], ps[:])
            nc.sync.dma_start(pin[:], psc[:])
            nc.gpsimd.collective_compute(
                "AllGather", Alu.bypass, replica_groups=shard_groups,
                ins=[pin[:]], outs=[pall[:]],
            )
            pall_sb = prep.tile([SH, NGROUP * BC], f32, tag="pall_sb")
            nc.sync.dma_start(pall_sb[:], pall[:])
            ones4 = prep.tile([SH, 1], f32, tag="ones4")
            nc.vector.memset(ones4[:], 1.0)
            osb = prep.tile([1, NGROUP * BC], f32, tag="osb")
            for k in range(2):
                ps2 = psum.tile([1, HW], f32, tag=f"ps2{k}")
                nc.tensor.matmul(
                    ps2[:], ones4[:], pall_sb[:, k * HW : (k + 1) * HW],
                    start=True, stop=True,
                )
                nc.scalar.mul(osb[:, k * HW : (k + 1) * HW], ps2[:], 1.0 / TAU)
            # consume the warm-up collective's (all-zero) output so DCE keeps it
            wsb2 = prep.tile([1, 16], f32, tag="wsb2")
            nc.sync.dma_start(wsb2[:], warm[0:1, :])
            nc.vector.tensor_add(osb[:, :16], osb[:, :16], wsb2[:])
            nc.sync.dma_start(out_d[:], osb[:])

    nc.compile()
    return nc


def _wrap_idx(ii):
    w = ii.astype(np.int16).reshape(-1, 16).T
    return np.ascontiguousarray(np.tile(w, (8, 1)))


CHUNKS12 = [(0, 8), (8, 16), (16, 24), (24, 32)]
CHUNKS3 = [(0, 11), (11, 22), (22, 33)]


def _combine_idx(ia_eff, ib_eff, chunk_list):
    """Interleave a/b index streams per chunk: [a-chunk0, b-chunk0, a-chunk1, ...]"""
    parts = []
    for j0, j1 in chunk_list:
        parts.append(ia_eff[j0 * P : j1 * P])
        parts.append(ib_eff[j0 * P : j1 * P])
    return _wrap_idx(np.concatenate(parts))


def _pack_w(w_eff, nj):
    # local slot t = j*128 + p  ->  packed[p, j*16+g]
    return np.ascontiguousarray(
        w_eff.reshape(nj, P, 16).transpose(1, 0, 2).reshape(P, nj * 16)
    )


NCH = 4
JCH = NJ12 // NCH


def _src_unit12(i):
    """BC-row unit of layer-1/2 neuron i in the chunk-major AllGathered
    [NCH*SH*128, JCH*BC] layout: shard s = i//4000, local t = i - 4000s,
    p = t%128, j = t//128, chunk k = j//JCH; row = (k*SH+s)*128+p,
    unit = row*JCH + j%JCH."""
    s = i // REAL12
    t = i - s * REAL12
    p = t % P
    j = t // P
    k = j // JCH
    return ((k * SH + s) * P + p) * JCH + j % JCH


def _host_pack(inputs):
    x = np.asarray(inputs["x"], dtype=np.float32)
    w1 = np.asarray(inputs["w1"], dtype=np.float32)
    w2 = np.asarray(inputs["w2"], dtype=np.float32)
    w3 = np.asarray(inputs["w3"], dtype=np.float32)
    i1a = np.asarray(inputs["idx1a"]).astype(np.int64)
    i1b = np.asarray(inputs["idx1b"]).astype(np.int64)
    i2a = np.asarray(inputs["idx2a"]).astype(np.int64)
    i2b = np.asarray(inputs["idx2b"]).astype(np.int64)
    i3a = np.asarray(inputs["idx3a"]).astype(np.int64)
    i3b = np.asarray(inputs["idx3b"]).astype(np.int64)

    pad_row = np.full(16, -20.0, dtype=np.float32)
    pad_row[0] = 20.0  # softmax -> ~one-hot FALSE gate -> h = 0

    per_shard = [dict() for _ in range(SH)]
    # layers 1 and 2: shard s owns real neurons [s*4000, (s+1)*4000)
    for l, (w, ja, jb, srcf) in enumerate(
        (
            (w1, i1a, i1b, lambda i: i),
            (w2, i2a, i2b, _src_unit12),
        ),
        start=1,
    ):
        for s in range(SH):
            sel = slice(s * REAL12, (s + 1) * REAL12)
            w_eff = np.concatenate(
                [w[sel], np.tile(pad_row, (NS12 - REAL12, 1))], axis=0
            )
            ia_eff = np.zeros(NS12, dtype=np.int64)
            ib_eff = np.zeros(NS12, dtype=np.int64)
            ia_eff[:REAL12] = srcf(ja[sel])
            ib_eff[:REAL12] = srcf(jb[sel])
            per_shard[s][f"w{l}p"] = _pack_w(w_eff, NJ12)
            per_shard[s][f"i{l}"] = _combine_idx(ia_eff, ib_eff, CHUNKS12)

    # layer 3: group g's 5333 real neurons split over shards as
    # counts c_s = [1334, 1333, 1333, 1333]; within (s, g): local j in
    # [11g, 11g+11), rank m = (j-11g)*128 + p
    counts = np.array([1334, 1333, 1333, 1333])
    offs = np.concatenate([[0], np.cumsum(counts)[:-1]])
    u = np.arange(NS3)
    jj = u // P
    pp = u % P
    gg = jj // JPG
    m = (jj - gg * JPG) * P + pp
    for s in range(SH):
        real = m < counts[s]
        rid = gg * SPG + offs[s] + np.minimum(m, counts[s] - 1)
        w3_eff = w3[rid].copy()
        w3_eff[~real] = pad_row
        i3a_eff = np.where(real, _src_unit12(i3a[rid]), 0)
        i3b_eff = np.where(real, _src_unit12(i3b[rid]), 0)
        per_shard[s]["w3p"] = _pack_w(w3_eff, NJ3)
        per_shard[s]["i3"] = _combine_idx(i3a_eff, i3b_eff, CHUNKS3)

    in_maps = []
    for c in range(N_CORES):
        G, s = c // SH, c % SH
        m_ = dict(per_shard[s])
        m_["xT"] = np.ascontiguousarray(x[G * BC : (G + 1) * BC].T)
        in_maps.append(m_)
    return in_maps


LAST_RESULTS = None


def kernel(**inputs):
    global LAST_RESULTS
    from concourse.bass_utils import run_bass_kernel_spmd

    if "nc" not in _CACHE:
        _CACHE["nc"] = _build_nc()
    nc = _CACHE["nc"]

    in_maps = _host_pack(inputs)
    trace = bool(int(os.environ.get("KERNEL_TRACE", "0")))
    res = run_bass_kernel_spmd(
        nc, in_maps, core_ids=list(range(N_CORES)), trace=trace
    )
    LAST_RESULTS = res

    out = np.empty((B, NGROUP), dtype=np.float32)
    for g_ in range(BG):
        rc = res.results[g_ * SH]["out"].reshape(NGROUP, BC)
        out[g_ * BC : (g_ + 1) * BC, :] = rc.T
    return out



# revision 5
# speedup vs baseline: 1.1519x; 1.1519x over previous
"""Trainium2 Bass kernel for a 3-layer difflogic network (nn_Net_48610439856713).

Math: each layer o computes softmax(w[o])·ops16(a, b) with a = h[:, ia[o]],
b = h[:, ib[o]].  The 16 relaxed logic gates are all affine in {1, a, b, ab},
so the layer reduces to  h' = C0 + C1·a + C2·b + C3·a·b  with 4 per-neuron
coefficients derived on-device from softmax(w).

v3 design (vs the 2x4 baseline):
  - 8-way neuron sharding, full batch B=512 per core.  Layer exchange is ONE
    8-rank AllGather per layer (RDH algorithm, ~66us for 16MB out), which
    beats any 4-rank collective shape on this chip.
  - Layer 1 is computed as one-hot matmuls on the (otherwise idle) TensorE:
    a/b operand rows land in PSUM, removing layer-1's dma_gather descriptor
    generation (the Q7 SWDGE desc-gen at ~8ns/index is a main bottleneck).
  - Layer 2/3 gathers use prepare_only=True dma_gather: Q7 descriptor
    generation runs EARLY (overlapped with layer-1 compute / AllGather-1),
    and trigger_dma fires the prepared DMA once the AllGather has landed.
    A probe DMA reading the AG output + a WAW dep via signals_writable
    pins each trigger after its collective completes (the rust-side dep
    deferral alone gates only on the collective *doorbell*).
  - The per-neuron affine combine runs as 6 whole-layer DVE ops using
    stride-0 broadcast APs for the coefficient operands (instead of 3 small
    scalar_tensor_tensor ops per 128-slot column).

Host-side bookkeeping is integer/layout only: slot permutations, index
relabeling through the packed layout, int16 index wrapping, weight-row
packing, one-hot matrix construction.  All float arithmetic (softmax,
combine, sums) runs on device.
"""

import os
import numpy as np

P = 128
B = 512                  # full batch on every core
IN = 193
NGROUP = 3
TAU = 100.0
N_CORES = 8
SH = 8                   # neuron shards
BG = 1
BC = B                   # batch per core

NJ12 = 16                # j-columns per shard, layers 1/2
REAL12 = 2000            # real neurons per shard, layers 1/2
NS12 = NJ12 * P          # 2048 slots per shard

NJ3 = 18                 # layer 3: 3 groups x 6 j-cols
JPG = 6
NS3 = NJ3 * P            # 2304 slots
SPG = 15999 // NGROUP    # 5333 real neurons per group
CNT3 = [667, 667, 667, 667, 667, 666, 666, 666]   # per-shard split of 5333
OFF3 = np.concatenate([[0], np.cumsum(CNT3)[:-1]])

HD = os.environ.get("KERNEL_HDT", "bf16")   # exchanged-activation dtype

_CACHE = {}


def _build_nc():
    import concourse.bacc as bacc
    import concourse.tile as tile
    import concourse.mybir as mybir

    f32 = mybir.dt.float32
    bf16 = mybir.dt.bfloat16
    i16 = mybir.dt.int16
    hdt = mybir.dt.float8e4 if HD == "f8" else bf16
    Alu = mybir.AluOpType
    Act = mybir.ActivationFunctionType
    Ax = mybir.AxisListType

    nc = bacc.Bacc("TRN2", target_bir_lowering=False, debug=False,
                   num_devices=N_CORES)

    G8 = [[0, 1, 2, 3, 4, 5, 6, 7]]

    # ---- I/O ----
    xT = nc.dram_tensor("xT", [IN, B], f32, kind="ExternalInput")
    oh0 = nc.dram_tensor("oh0", [P, 2 * NS12], bf16, kind="ExternalInput")
    oh1 = nc.dram_tensor("oh1", [IN - P, 2 * NS12], bf16, kind="ExternalInput")
    wps = [
        nc.dram_tensor("w1p", [P, NJ12 * 16], f32, kind="ExternalInput"),
        nc.dram_tensor("w2p", [P, NJ12 * 16], f32, kind="ExternalInput"),
        nc.dram_tensor("w3p", [P, NJ3 * 16], f32, kind="ExternalInput"),
    ]
    i2d = nc.dram_tensor("i2", [P, 2 * NS12 // 16], i16, kind="ExternalInput")
    i3d = nc.dram_tensor("i3", [P, 2 * NS3 // 16], i16, kind="ExternalInput")
    out_d = nc.dram_tensor("out", [1, NGROUP * B], f32, kind="ExternalOutput")

    # collective buffers
    cin = [nc.dram_tensor(f"cin{l}", [P, NJ12 * B], hdt, kind="Internal")
           for l in (1, 2)]
    gs_ = [nc.dram_tensor(f"g{l}", [SH * P, NJ12 * B], hdt, kind="Internal",
                          addr_space="Shared")
           for l in (1, 2)]
    win = nc.dram_tensor("win", [1, 16], f32, kind="Internal")
    warm = nc.dram_tensor("warm", [8, 16], f32, kind="Internal",
                          addr_space="Shared")
    pin = nc.dram_tensor("pin", [1, NGROUP * B], f32, kind="Internal")
    pall = nc.dram_tensor("pall", [8, NGROUP * B], f32, kind="Internal",
                          addr_space="Shared")

    def coeffs(pool, wp, nj, li):
        """softmax(w) -> affine coefficients C0..C3, each [P, nj] f32."""
        tg = lambda name: f"{name}_{li}"
        wt = pool.tile([P, nj * 16], f32, tag=tg("wt"))
        nc.sync.dma_start(wt[:], wp[:])
        e = pool.tile([P, nj * 16], f32, tag=tg("e"))
        nc.scalar.activation(e[:], wt[:], Act.Exp)
        e3 = e[:].rearrange("p (j g) -> p j g", g=16)
        e4 = e[:].rearrange("p (j h q) -> p j h q", h=4, q=4)

        ssum = pool.tile([P, nj], f32, tag=tg("ssum"))
        nc.vector.reduce_sum(ssum[:], e3, axis=Ax.X)
        r = pool.tile([P, nj], f32, tag=tg("r"))
        nc.vector.reciprocal(r[:], ssum[:])

        c0 = pool.tile([P, nj], f32, tag=tg("c0"))
        c1 = pool.tile([P, nj], f32, tag=tg("c1"))
        c2 = pool.tile([P, nj], f32, tag=tg("c2"))
        c3 = pool.tile([P, nj], f32, tag=tg("c3"))

        nc.vector.reduce_sum(c0[:], e4[:, :, 2:4, :], axis=Ax.XY)
        t1 = pool.tile([P, nj], f32, tag=tg("t1"))
        t2 = pool.tile([P, nj], f32, tag=tg("t2"))
        nc.vector.reduce_sum(t1[:], e4[:, :, 0:2, 2:4], axis=Ax.XY)
        nc.vector.reduce_sum(t2[:], e4[:, :, 2:4, 0:2], axis=Ax.XY)
        nc.vector.tensor_sub(c1[:], t1[:], t2[:])
        nc.vector.reduce_sum(t1[:], e4[:, :, 1, :], axis=Ax.X)
        nc.vector.reduce_sum(t2[:], e4[:, :, 2, :], axis=Ax.X)
        nc.vector.tensor_sub(c2[:], t1[:], t2[:])
        f = pool.tile([P, nj, 7], f32, tag=tg("f"))
        nc.vector.tensor_sub(f[:], e3[:, :, 1:8], e3[:, :, 14:7:-1])
        u1 = pool.tile([P, nj], f32, tag=tg("u1"))
        u2 = pool.tile([P, nj], f32, tag=tg("u2"))
        nc.vector.tensor_sub(u1[:], f[:, :, 0], f[:, :, 1])
        nc.vector.tensor_add(u2[:], f[:, :, 3], f[:, :, 6])
        nc.vector.tensor_sub(u1[:], u1[:], u2[:])
        nc.vector.scalar_tensor_tensor(
            c3[:], f[:, :, 5], -2.0, u1[:], op0=Alu.mult, op1=Alu.add
        )
        for ck in (c0, c1, c2, c3):
            nc.vector.tensor_mul(ck[:], ck[:], r[:])
        return c0, c1, c2, c3

    def combine(a3, b3, cs, hout, tsl, usl, nj, bc):
        """hout = C0 + C1 a + C2 b + C3 ab over [P, nj, bc] via broadcast."""
        c0, c1, c2, c3 = cs
        shp = [P, nj, bc]
        c0b = c0[:].unsqueeze(2).broadcast_to(shp)
        c1b = c1[:].unsqueeze(2).broadcast_to(shp)
        c2b = c2[:].unsqueeze(2).broadcast_to(shp)
        c3b = c3[:].unsqueeze(2).broadcast_to(shp)
        nc.vector.tensor_mul(tsl, a3, c3b)      # C3 a
        nc.vector.tensor_add(tsl, tsl, c2b)     # C3 a + C2
        nc.vector.tensor_mul(tsl, tsl, b3)      # C3 ab + C2 b
        nc.vector.tensor_mul(usl, a3, c1b)      # C1 a
        nc.vector.tensor_add(usl, usl, c0b)     # C1 a + C0
        nc.vector.tensor_add(hout, tsl, usl)

    with tile.TileContext(nc) as tc:
        with (
            tc.tile_pool(name="big", bufs=1) as big,
            tc.tile_pool(name="prep", bufs=1) as prep,
            tc.tile_pool(name="small", bufs=2) as small,
        ):
            ds2 = nc.alloc_semaphore("ds2")
            ds3 = nc.alloc_semaphore("ds3")

            # shared combine scratch (sliced per layer)
            ct = big.tile([P, NJ3, B], bf16, tag="ct")
            cu = big.tile([P, NJ3, B], bf16, tag="cu")

            # ---- warm-up collective (absorbs first-collective barrier) ----
            wsb = small.tile([1, 16], f32, tag="wsb")
            nc.vector.memset(wsb[:], 0.0)
            nc.sync.dma_start(win[:], wsb[:])
            nc.gpsimd.collective_compute(
                "AllGather", Alu.bypass, replica_groups=G8,
                ins=[win[:]], outs=[warm[:]],
            )

            # ---- L2 gather prep (desc-gen runs now; DMA fires at trigger) ----
            i2t = small.tile([P, 2 * NS12 // 16], i16, tag="i2t")
            nc.sync.dma_start(i2t[:], i2d[:])
            ab2 = big.tile([P, 2 * NJ12, B], hdt, tag="ab2")
            g1rows = gs_[0][:].rearrange("r (j b) -> (r j) b", b=B)
            nc.gpsimd.dma_gather(
                ab2[:], g1rows, i2t[:], 2 * NS12, 2 * NS12, B,
                prepare_only=True, sem=ds2, single_packet=False,
            )

            # ---- layer 1: one-hot matmuls + combine ----
            cs1 = coeffs(prep, wps[0], NJ12, 1)
            h1 = big.tile([P, NJ12, B], hdt, tag="h")

            with tc.tile_pool(name="l1", bufs=1) as l1p:
                xs0 = l1p.tile([P, B], f32, tag="xs0")
                xs1 = l1p.tile([IN - P, B], f32, tag="xs1")
                nc.sync.dma_start(xs0[:], xT[0:P, :])
                nc.sync.dma_start(xs1[:], xT[P:IN, :])
                xb0 = l1p.tile([P, B], bf16, tag="xb0")
                xb1 = l1p.tile([IN - P, B], bf16, tag="xb1")
                nc.scalar.copy(xb0[:], xs0[:])
                nc.scalar.copy(xb1[:], xs1[:])
                oh0t = l1p.tile([P, 2 * NS12], bf16, tag="oh0t")
                oh1t = l1p.tile([IN - P, 2 * NS12], bf16, tag="oh1t")
                nc.sync.dma_start(oh0t[:], oh0[:])
                nc.sync.dma_start(oh1t[:], oh1[:])

                JC = 2  # j-columns per PSUM chunk (2 banks each for a and b)
                with tc.tile_pool(name="ps1", bufs=2, space="PSUM") as ps1:
                    for jc in range(0, NJ12, JC):
                        pa = ps1.tile([P, JC, B], f32, tag="pa")
                        pb = ps1.tile([P, JC, B], f32, tag="pb")
                        for jj in range(JC):
                            j = jc + jj
                            ca = j * P
                            cb = NS12 + j * P
                            nc.tensor.matmul(pa[:, jj], oh0t[:, ca:ca + P],
                                             xb0[:], start=True, stop=False)
                            nc.tensor.matmul(pa[:, jj], oh1t[:, ca:ca + P],
                                             xb1[:], start=False, stop=True)
                            nc.tensor.matmul(pb[:, jj], oh0t[:, cb:cb + P],
                                             xb0[:], start=True, stop=False)
                            nc.tensor.matmul(pb[:, jj], oh1t[:, cb:cb + P],
                                             xb1[:], start=False, stop=True)
                        csl = tuple(c[:, jc:jc + JC] for c in cs1)
                        combine(pa[:], pb[:], csl, h1[:, jc:jc + JC, :],
                                ct[:, jc:jc + JC, :], cu[:, jc:jc + JC, :],
                                JC, B)

            nc.sync.dma_start(cin[0][:], h1[:].rearrange("p j b -> p (j b)"))
            nc.gpsimd.collective_compute(
                "AllGather", Alu.bypass, replica_groups=G8,
                ins=[cin[0][:]], outs=[gs_[0][:]],
            )

            # ---- trigger L2 gather once g1 has landed ----
            pr1 = small.tile([1, 64], hdt, tag="pr1")
            nc.sync.dma_start(pr1[:], gs_[0][0:1, 0:64])
            nc.gpsimd.trigger_dma(count=None, signals_writable=[pr1[:]])

            # ---- L3 gather prep (desc-gen after trigger2 in program order) ----
            i3t = small.tile([P, 2 * NS3 // 16], i16, tag="i3t")
            nc.sync.dma_start(i3t[:], i3d[:])
            ab3 = big.tile([P, 2 * NJ3, B], hdt, tag="ab3")
            g2rows = gs_[1][:].rearrange("r (j b) -> (r j) b", b=B)
            nc.gpsimd.dma_gather(
                ab3[:], g2rows, i3t[:], 2 * NS3, 2 * NS3, B,
                prepare_only=True, sem=ds3, single_packet=False,
            )

            # ---- layer 2 ----
            cs2 = coeffs(prep, wps[1], NJ12, 2)
            h2 = big.tile([P, NJ12, B], hdt, tag="h")
            nc.vector.wait_ge(ds2, 16)  # gather DMA (fired by trigger) landed
            combine(ab2[:, 0:NJ12, :], ab2[:, NJ12:2 * NJ12, :], cs2, h2[:],
                    ct[:, 0:NJ12, :], cu[:, 0:NJ12, :], NJ12, B)
            nc.sync.dma_start(cin[1][:], h2[:].rearrange("p j b -> p (j b)"))
            nc.gpsimd.collective_compute(
                "AllGather", Alu.bypass, replica_groups=G8,
                ins=[cin[1][:]], outs=[gs_[1][:]],
            )

            # ---- trigger L3 gather ----
            pr2 = small.tile([1, 64], hdt, tag="pr2")
            nc.sync.dma_start(pr2[:], gs_[1][0:1, 0:64])
            nc.gpsimd.trigger_dma(count=None, signals_writable=[pr2[:]])

            # ---- layer 3 ----
            cs3 = coeffs(prep, wps[2], NJ3, 3)
            h3 = big.tile([P, NJ3, B], bf16, tag="h3")
            nc.vector.wait_ge(ds3, 16)  # gather DMA (fired by trigger) landed
            combine(ab3[:, 0:NJ3, :], ab3[:, NJ3:2 * NJ3, :], cs3, h3[:],
                    ct[:], cu[:], NJ3, B)

            # ---- GroupSum ----
            gsum = prep.tile([P, NGROUP * B], f32, tag="gsum")
            for g in range(NGROUP):
                sl = h3[:, g * JPG:(g + 1) * JPG, :].rearrange("p j b -> p b j")
                nc.vector.reduce_sum(gsum[:, g * B:(g + 1) * B], sl, axis=Ax.X)
            ones = small.tile([P, 1], f32, tag="ones")
            nc.vector.memset(ones[:], 1.0)
            psc = prep.tile([1, NGROUP * B], f32, tag="psc")
            with tc.tile_pool(name="ps2", bufs=2, space="PSUM") as ps2:
                for g in range(NGROUP):
                    ps = ps2.tile([1, B], f32, tag="psg")
                    nc.tensor.matmul(ps[:], ones[:], gsum[:, g * B:(g + 1) * B],
                                     start=True, stop=True)
                    nc.scalar.copy(psc[:, g * B:(g + 1) * B], ps[:])
                nc.sync.dma_start(pin[:], psc[:])
                nc.gpsimd.collective_compute(
                    "AllGather", Alu.bypass, replica_groups=G8,
                    ins=[pin[:]], outs=[pall[:]],
                )
                pall_sb = prep.tile([8, NGROUP * B], f32, tag="pall_sb")
                nc.sync.dma_start(pall_sb[:], pall[:])
                ones8 = small.tile([8, 1], f32, tag="ones8")
                nc.vector.memset(ones8[:], 1.0)
                osb = prep.tile([1, NGROUP * B], f32, tag="osb")
                for g in range(NGROUP):
                    ps2t = ps2.tile([1, B], f32, tag="psg2")
                    nc.tensor.matmul(ps2t[:], ones8[:],
                                     pall_sb[:, g * B:(g + 1) * B],
                                     start=True, stop=True)
                    nc.scalar.mul(osb[:, g * B:(g + 1) * B], ps2t[:], 1.0 / TAU)
            # consume the warm-up collective's (all-zero) output
            wsb2 = small.tile([1, 16], f32, tag="wsb2")
            nc.sync.dma_start(wsb2[:], warm[0:1, :])
            nc.vector.tensor_add(osb[:, :16], osb[:, :16], wsb2[:])
            nc.sync.dma_start(out_d[:], osb[:])

    nc.compile()
    return nc


def _wrap_idx(ii):
    w = ii.astype(np.int16).reshape(-1, 16).T
    return np.ascontiguousarray(np.tile(w, (8, 1)))


def _src_unit(i):
    """Row unit of layer-1/2 neuron i in the AllGathered [SH*128, NJ12*B]
    layout: shard s = i//2000, local t = i - 2000s, p = t%128, j = t//128;
    row = s*128+p, unit = row*NJ12 + j."""
    s = i // REAL12
    t = i - s * REAL12
    p = t % P
    j = t // P
    return (s * P + p) * NJ12 + j


def _pack_w(w_eff, nj):
    # local slot t = j*128 + p  ->  packed[p, j*16+g]
    return np.ascontiguousarray(
        w_eff.reshape(nj, P, 16).transpose(1, 0, 2).reshape(P, nj * 16)
    )


def _host_pack(inputs):
    x = np.asarray(inputs["x"], dtype=np.float32)
    w1 = np.asarray(inputs["w1"], dtype=np.float32)
    w2 = np.asarray(inputs["w2"], dtype=np.float32)
    w3 = np.asarray(inputs["w3"], dtype=np.float32)
    i1a = np.asarray(inputs["idx1a"]).astype(np.int64)
    i1b = np.asarray(inputs["idx1b"]).astype(np.int64)
    i2a = np.asarray(inputs["idx2a"]).astype(np.int64)
    i2b = np.asarray(inputs["idx2b"]).astype(np.int64)
    i3a = np.asarray(inputs["idx3a"]).astype(np.int64)
    i3b = np.asarray(inputs["idx3b"]).astype(np.int64)

    import ml_dtypes

    pad_row = np.full(16, -20.0, dtype=np.float32)
    pad_row[0] = 20.0  # softmax -> ~one-hot FALSE gate -> h = 0

    xTf = np.ascontiguousarray(x.T)  # [193, 512]

    per_shard = []
    for s in range(SH):
        m = {}
        sel = slice(s * REAL12, (s + 1) * REAL12)

        # layer 1: one-hot matrices + packed weights
        w1_eff = np.concatenate(
            [w1[sel], np.tile(pad_row, (NS12 - REAL12, 1))], axis=0
        )
        m["w1p"] = _pack_w(w1_eff, NJ12)
        oh = np.zeros((IN, 2 * NS12), dtype=ml_dtypes.bfloat16)
        cols = np.arange(REAL12)
        oh[i1a[sel], cols] = 1.0
        oh[i1b[sel], NS12 + cols] = 1.0
        m["oh0"] = np.ascontiguousarray(oh[0:P])
        m["oh1"] = np.ascontiguousarray(oh[P:IN])

        # layer 2
        w2_eff = np.concatenate(
            [w2[sel], np.tile(pad_row, (NS12 - REAL12, 1))], axis=0
        )
        m["w2p"] = _pack_w(w2_eff, NJ12)
        ia = np.zeros(NS12, dtype=np.int64)
        ib = np.zeros(NS12, dtype=np.int64)
        ia[:REAL12] = _src_unit(i2a[sel])
        ib[:REAL12] = _src_unit(i2b[sel])
        m["i2"] = _wrap_idx(np.concatenate([ia, ib]))

        # layer 3: group g's 5333 real neurons split per CNT3; within (s, g):
        # local j in [6g, 6g+6), rank m = (j-6g)*128 + p
        u = np.arange(NS3)
        jj = u // P
        pp = u % P
        gg = jj // JPG
        mm = (jj - gg * JPG) * P + pp
        real = mm < CNT3[s]
        rid = gg * SPG + OFF3[s] + np.minimum(mm, CNT3[s] - 1)
        w3_eff = w3[rid].copy()
        w3_eff[~real] = pad_row
        m["w3p"] = _pack_w(w3_eff, NJ3)
        i3a_eff = np.where(real, _src_unit(i3a[rid]), 0)
        i3b_eff = np.where(real, _src_unit(i3b[rid]), 0)
        m["i3"] = _wrap_idx(np.concatenate([i3a_eff, i3b_eff]))

        m["xT"] = xTf
        per_shard.append(m)
    return per_shard


LAST_RESULTS = None


def kernel(**inputs):
    global LAST_RESULTS
    from concourse.bass_utils import run_bass_kernel_spmd

    if "nc" not in _CACHE:
        _CACHE["nc"] = _build_nc()
    nc = _CACHE["nc"]

    in_maps = _host_pack(inputs)
    trace = bool(int(os.environ.get("KERNEL_TRACE", "0")))
    res = run_bass_kernel_spmd(
        nc, in_maps, core_ids=list(range(N_CORES)), trace=trace
    )
    LAST_RESULTS = res

    rc = res.results[0]["out"].reshape(NGROUP, B)
    return np.ascontiguousarray(rc.T.astype(np.float32))


# revision 13
# speedup vs baseline: 1.3354x; 1.1593x over previous
"""Trainium2 Bass kernel for a 3-layer difflogic network (nn_Net_48610439856713).

Math: each layer o computes softmax(w[o])·ops16(a, b) with a = h[:, ia[o]],
b = h[:, ib[o]].  The 16 relaxed logic gates are all affine in {1, a, b, ab},
so the layer reduces to  h' = C0 + C1·a + C2·b + C3·a·b  with 4 per-neuron
coefficients derived on-device from softmax(w).

v3 design (vs the 2x4 baseline):
  - 8-way neuron sharding, full batch B=512 per core.  Layer exchange is ONE
    8-rank AllGather per layer (RDH algorithm, ~66us for 16MB out), which
    beats any 4-rank collective shape on this chip.
  - Layer 1 is computed as one-hot matmuls on the (otherwise idle) TensorE:
    a/b operand rows land in PSUM, removing layer-1's dma_gather descriptor
    generation (the Q7 SWDGE desc-gen at ~8ns/index is a main bottleneck).
  - Layer 2/3 gathers use prepare_only=True dma_gather: Q7 descriptor
    generation runs EARLY (overlapped with layer-1 compute / AllGather-1),
    and trigger_dma fires the prepared DMA once the AllGather has landed.
    A probe DMA reading the AG output + a WAW dep via signals_writable
    pins each trigger after its collective completes (the rust-side dep
    deferral alone gates only on the collective *doorbell*).
  - The per-neuron affine combine runs as 6 whole-layer DVE ops using
    stride-0 broadcast APs for the coefficient operands (instead of 3 small
    scalar_tensor_tensor ops per 128-slot column).

Host-side bookkeeping is integer/layout only: slot permutations, index
relabeling through the packed layout, int16 index wrapping, weight-row
packing, one-hot matrix construction.  All float arithmetic (softmax,
combine, sums) runs on device.
"""

import os
import numpy as np

P = 128
B = 512                  # full batch on every core
IN = 193
NGROUP = 3
TAU = 100.0
N_CORES = 8
SH = 8                   # neuron shards
BG = 1
BC = B                   # batch per core

NJ12 = 16                # j-columns per shard, layers 1/2
REAL12 = 2000            # real neurons per shard, layers 1/2
NS12 = NJ12 * P          # 2048 slots per shard

NJ3 = 18                 # layer 3: 3 groups x 6 j-cols
JPG = 6
NS3 = NJ3 * P            # 2304 slots
SPG = 15999 // NGROUP    # 5333 real neurons per group
CNT3 = [667, 667, 667, 667, 667, 666, 666, 666]   # per-shard split of 5333
OFF3 = np.concatenate([[0], np.cumsum(CNT3)[:-1]])

HD = os.environ.get("KERNEL_HDT", "f8")   # exchanged-activation dtype
SP = bool(int(os.environ.get("KERNEL_SP", "0")))  # single_packet gathers

_CACHE = {}


def _build_nc():
    import concourse.bacc as bacc
    import concourse.tile as tile
    import concourse.mybir as mybir

    f32 = mybir.dt.float32
    bf16 = mybir.dt.bfloat16
    i16 = mybir.dt.int16
    hdt = mybir.dt.float8e4 if HD == "f8" else bf16
    Alu = mybir.AluOpType
    Act = mybir.ActivationFunctionType
    Ax = mybir.AxisListType

    nc = bacc.Bacc("TRN2", target_bir_lowering=False, debug=False,
                   num_devices=N_CORES)

    G8 = [[0, 1, 2, 3, 4, 5, 6, 7]]

    # ---- I/O ----
    xT = nc.dram_tensor("xT", [IN, B], f32, kind="ExternalInput")
    oh0 = nc.dram_tensor("oh0", [P, 2 * NS12], bf16, kind="ExternalInput")
    oh1 = nc.dram_tensor("oh1", [IN - P, 2 * NS12], bf16, kind="ExternalInput")
    wps = [
        nc.dram_tensor("w1p", [P, NJ12 * 16], f32, kind="ExternalInput"),
        nc.dram_tensor("w2p", [P, NJ12 * 16], f32, kind="ExternalInput"),
        nc.dram_tensor("w3p", [P, NJ3 * 16], f32, kind="ExternalInput"),
    ]
    i2d = nc.dram_tensor("i2", [P, 2 * NS12 // 16], i16, kind="ExternalInput")
    i3d = nc.dram_tensor("i3", [P, 2 * NS3 // 16], i16, kind="ExternalInput")
    out_d = nc.dram_tensor("out", [1, NGROUP * B], f32, kind="ExternalOutput")

    # collective buffers
    cin = [nc.dram_tensor(f"cin{l}", [P, NJ12 * B], hdt, kind="Internal")
           for l in (1, 2)]
    gs_ = [nc.dram_tensor(f"g{l}", [SH * P, NJ12 * B], hdt, kind="Internal",
                          addr_space="Shared")
           for l in (1, 2)]
    win = nc.dram_tensor("win", [1, 16], f32, kind="Internal")
    warm = nc.dram_tensor("warm", [8, 16], f32, kind="Internal",
                          addr_space="Shared")
    pin = nc.dram_tensor("pin", [1, NGROUP * B], f32, kind="Internal")
    pall = nc.dram_tensor("pall", [8, NGROUP * B], f32, kind="Internal",
                          addr_space="Shared")

    def coeffs(pool, wp, nj, li):
        """softmax(w) -> affine coefficients C0..C3, each [P, nj] f32."""
        tg = lambda name: f"{name}_{li}"
        wt = pool.tile([P, nj * 16], f32, tag=tg("wt"))
        nc.sync.dma_start(wt[:], wp[:])
        e = pool.tile([P, nj * 16], f32, tag=tg("e"))
        nc.scalar.activation(e[:], wt[:], Act.Exp)
        e3 = e[:].rearrange("p (j g) -> p j g", g=16)
        e4 = e[:].rearrange("p (j h q) -> p j h q", h=4, q=4)

        ssum = pool.tile([P, nj], f32, tag=tg("ssum"))
        nc.vector.reduce_sum(ssum[:], e3, axis=Ax.X)
        r = pool.tile([P, nj], f32, tag=tg("r"))
        nc.vector.reciprocal(r[:], ssum[:])

        c0 = pool.tile([P, nj], f32, tag=tg("c0"))
        c1 = pool.tile([P, nj], f32, tag=tg("c1"))
        c2 = pool.tile([P, nj], f32, tag=tg("c2"))
        c3 = pool.tile([P, nj], f32, tag=tg("c3"))

        nc.vector.reduce_sum(c0[:], e4[:, :, 2:4, :], axis=Ax.XY)
        t1 = pool.tile([P, nj], f32, tag=tg("t1"))
        t2 = pool.tile([P, nj], f32, tag=tg("t2"))
        nc.vector.reduce_sum(t1[:], e4[:, :, 0:2, 2:4], axis=Ax.XY)
        nc.vector.reduce_sum(t2[:], e4[:, :, 2:4, 0:2], axis=Ax.XY)
        nc.vector.tensor_sub(c1[:], t1[:], t2[:])
        nc.vector.reduce_sum(t1[:], e4[:, :, 1, :], axis=Ax.X)
        nc.vector.reduce_sum(t2[:], e4[:, :, 2, :], axis=Ax.X)
        nc.vector.tensor_sub(c2[:], t1[:], t2[:])
        f = pool.tile([P, nj, 7], f32, tag=tg("f"))
        nc.vector.tensor_sub(f[:], e3[:, :, 1:8], e3[:, :, 14:7:-1])
        u1 = pool.tile([P, nj], f32, tag=tg("u1"))
        u2 = pool.tile([P, nj], f32, tag=tg("u2"))
        nc.vector.tensor_sub(u1[:], f[:, :, 0], f[:, :, 1])
        nc.vector.tensor_add(u2[:], f[:, :, 3], f[:, :, 6])
        nc.vector.tensor_sub(u1[:], u1[:], u2[:])
        nc.vector.scalar_tensor_tensor(
            c3[:], f[:, :, 5], -2.0, u1[:], op0=Alu.mult, op1=Alu.add
        )
        for ck in (c0, c1, c2, c3):
            nc.vector.tensor_mul(ck[:], ck[:], r[:])
        # bf16 copies: an f32 broadcast operand forces the DVE combine ops
        # out of 16-bit 2x mode, doubling their cost
        cbs = []
        for nm, ck in (("b0", c0), ("b1", c1), ("b2", c2), ("b3", c3)):
            cb = pool.tile([P, nj], bf16, tag=tg(nm))
            nc.scalar.copy(cb[:], ck[:])
            cbs.append(cb)
        return tuple(cbs)

    def combine(a3, b3, cs, hout, tsl, usl, nj, bc, dsem=None):
        """hout = C0 + C1 a + C2 b + C3 ab over [P, nj, bc] via broadcast.

        dsem: DMA-completion semaphore of the prepared gather that produced
        a3/b3; attached as a wait on the ops that read them (a bare engine
        wait_ge instruction has no data deps, so the Tile scheduler is free
        to hoist it into a deadlock)."""
        c0, c1, c2, c3 = cs
        shp = [P, nj, bc]
        c0b = c0[:].unsqueeze(2).broadcast_to(shp)
        c1b = c1[:].unsqueeze(2).broadcast_to(shp)
        c2b = c2[:].unsqueeze(2).broadcast_to(shp)
        c3b = c3[:].unsqueeze(2).broadcast_to(shp)
        i1 = nc.vector.tensor_mul(tsl, a3, c3b)      # C3 a
        if dsem is not None:
            i1.wait_op(dsem, 16, "sem-ge")
        nc.vector.tensor_add(tsl, tsl, c2b)          # C3 a + C2
        nc.vector.tensor_mul(tsl, tsl, b3)           # C3 ab + C2 b
        i4 = nc.vector.tensor_mul(usl, a3, c1b)      # C1 a
        if dsem is not None:
            i4.wait_op(dsem, 16, "sem-ge")
        nc.vector.tensor_add(usl, usl, c0b)          # C1 a + C0
        nc.vector.tensor_add(hout, tsl, usl)

    with tile.TileContext(nc) as tc:
        with (
            tc.tile_pool(name="big", bufs=1) as big,
            tc.tile_pool(name="prep", bufs=1) as prep,
            tc.tile_pool(name="small", bufs=2) as small,
        ):
            ds2 = nc.alloc_semaphore("ds2")
            ds3 = nc.alloc_semaphore("ds3")

            # shared combine scratch (sliced per layer)
            ct = big.tile([P, NJ3, B], bf16, tag="ct")
            cu = big.tile([P, NJ3, B], bf16, tag="cu")

            # ---- warm-up collective (absorbs first-collective barrier) ----
            wsb = small.tile([1, 16], f32, tag="wsb")
            nc.vector.memset(wsb[:], 0.0)
            nc.sync.dma_start(win[:], wsb[:])
            nc.gpsimd.collective_compute(
                "AllGather", Alu.bypass, replica_groups=G8,
                ins=[win[:]], outs=[warm[:]],
            )

            # ---- L2 gather prep (desc-gen runs now; DMA fires at trigger) ----
            i2t = small.tile([P, 2 * NS12 // 16], i16, tag="i2t")
            nc.sync.dma_start(i2t[:], i2d[:])
            ab2 = big.tile([P, 2 * NJ12, B], hdt, tag="ab2")
            g1rows = gs_[0][:].rearrange("r (j b) -> (r j) b", b=B)
            nc.gpsimd.dma_gather(
                ab2[:], g1rows, i2t[:], 2 * NS12, 2 * NS12, B,
                prepare_only=True, sem=ds2, single_packet=SP,
            )

            # ---- layer 1: one-hot matmuls + combine ----
            cs1 = coeffs(prep, wps[0], NJ12, 1)
            h1 = big.tile([P, NJ12, B], hdt, tag="h")

            with tc.tile_pool(name="l1", bufs=1) as l1p:
                xs0 = l1p.tile([P, B], f32, tag="xs0")
                xs1 = l1p.tile([IN - P, B], f32, tag="xs1")
                nc.sync.dma_start(xs0[:], xT[0:P, :])
                nc.sync.dma_start(xs1[:], xT[P:IN, :])
                xb0 = l1p.tile([P, B], bf16, tag="xb0")
                xb1 = l1p.tile([IN - P, B], bf16, tag="xb1")
                nc.scalar.copy(xb0[:], xs0[:])
                nc.scalar.copy(xb1[:], xs1[:])
                oh0t = l1p.tile([P, 2 * NS12], bf16, tag="oh0t")
                oh1t = l1p.tile([IN - P, 2 * NS12], bf16, tag="oh1t")
                nc.sync.dma_start(oh0t[:], oh0[:])
                nc.sync.dma_start(oh1t[:], oh1[:])

                # a/b operand rows land in PSUM chunks, are evacuated to an
                # f8 SBUF tile on ScalarE (cast), then layer 1 combines on the
                # same whole-layer path as layers 2/3.
                ab1 = big.tile([P, 2 * NJ12, B], hdt, tag="ab1")
                JC = 2  # j-columns per PSUM chunk (2 banks each for a and b)
                with tc.tile_pool(name="ps1", bufs=2, space="PSUM") as ps1:
                    for jc in range(0, NJ12, JC):
                        pa = ps1.tile([P, JC, B], f32, tag="pa")
                        pb = ps1.tile([P, JC, B], f32, tag="pb")
                        for jj in range(JC):
                            j = jc + jj
                            ca = j * P
                            cb = NS12 + j * P
                            nc.tensor.matmul(pa[:, jj], oh0t[:, ca:ca + P],
                                             xb0[:], start=True, stop=False)
                            nc.tensor.matmul(pa[:, jj], oh1t[:, ca:ca + P],
                                             xb1[:], start=False, stop=True)
                            nc.tensor.matmul(pb[:, jj], oh0t[:, cb:cb + P],
                                             xb0[:], start=True, stop=False)
                            nc.tensor.matmul(pb[:, jj], oh1t[:, cb:cb + P],
                                             xb1[:], start=False, stop=True)
                        nc.scalar.copy(ab1[:, jc:jc + JC, :], pa[:])
                        nc.scalar.copy(ab1[:, NJ12 + jc:NJ12 + jc + JC, :],
                                       pb[:])
                combine(ab1[:, 0:NJ12, :], ab1[:, NJ12:2 * NJ12, :], cs1,
                        h1[:], ct[:, 0:NJ12, :], cu[:, 0:NJ12, :], NJ12, B)

            nc.sync.dma_start(cin[0][:], h1[:].rearrange("p j b -> p (j b)"))
            nc.gpsimd.collective_compute(
                "AllGather", Alu.bypass, replica_groups=G8,
                ins=[cin[0][:]], outs=[gs_[0][:]],
            )

            # ---- trigger L2 gather once g1 has landed ----
            pr1 = small.tile([1, 64], hdt, tag="pr1")
            nc.sync.dma_start(pr1[:], gs_[0][0:1, 0:64])
            nc.gpsimd.trigger_dma(count=None, signals_writable=[pr1[:]])

            # ---- L3 gather prep (desc-gen after trigger2 in program order) ----
            i3t = small.tile([P, 2 * NS3 // 16], i16, tag="i3t")
            nc.sync.dma_start(i3t[:], i3d[:])
            ab3 = big.tile([P, 2 * NJ3, B], hdt, tag="ab3")
            g2rows = gs_[1][:].rearrange("r (j b) -> (r j) b", b=B)
            nc.gpsimd.dma_gather(
                ab3[:], g2rows, i3t[:], 2 * NS3, 2 * NS3, B,
                prepare_only=True, sem=ds3, single_packet=SP,
            )

            # ---- layer 2 ----
            cs2 = coeffs(prep, wps[1], NJ12, 2)
            h2 = big.tile([P, NJ12, B], hdt, tag="h")
            combine(ab2[:, 0:NJ12, :], ab2[:, NJ12:2 * NJ12, :], cs2, h2[:],
                    ct[:, 0:NJ12, :], cu[:, 0:NJ12, :], NJ12, B, dsem=ds2)
            nc.sync.dma_start(cin[1][:], h2[:].rearrange("p j b -> p (j b)"))
            nc.gpsimd.collective_compute(
                "AllGather", Alu.bypass, replica_groups=G8,
                ins=[cin[1][:]], outs=[gs_[1][:]],
            )

            # ---- trigger L3 gather ----
            pr2 = small.tile([1, 64], hdt, tag="pr2")
            nc.sync.dma_start(pr2[:], gs_[1][0:1, 0:64])
            nc.gpsimd.trigger_dma(count=None, signals_writable=[pr2[:]])

            # ---- layer 3 ----
            cs3 = coeffs(prep, wps[2], NJ3, 3)
            h3 = big.tile([P, NJ3, B], bf16, tag="h3")
            combine(ab3[:, 0:NJ3, :], ab3[:, NJ3:2 * NJ3, :], cs3, h3[:],
                    ct[:], cu[:], NJ3, B, dsem=ds3)

            # ---- GroupSum ----
            gsum = prep.tile([P, NGROUP * B], f32, tag="gsum")
            for g in range(NGROUP):
                sl = h3[:, g * JPG:(g + 1) * JPG, :].rearrange("p j b -> p b j")
                nc.vector.reduce_sum(gsum[:, g * B:(g + 1) * B], sl, axis=Ax.X)
            ones = small.tile([P, 1], f32, tag="ones")
            nc.vector.memset(ones[:], 1.0)
            psc = prep.tile([1, NGROUP * B], f32, tag="psc")
            with tc.tile_pool(name="ps2", bufs=2, space="PSUM") as ps2:
                for g in range(NGROUP):
                    ps = ps2.tile([1, B], f32, tag="psg")
                    nc.tensor.matmul(ps[:], ones[:], gsum[:, g * B:(g + 1) * B],
                                     start=True, stop=True)
                    nc.scalar.copy(psc[:, g * B:(g + 1) * B], ps[:])
                nc.sync.dma_start(pin[:], psc[:])
                nc.gpsimd.collective_compute(
                    "AllGather", Alu.bypass, replica_groups=G8,
                    ins=[pin[:]], outs=[pall[:]],
                )
                pall_sb = prep.tile([8, NGROUP * B], f32, tag="pall_sb")
                nc.sync.dma_start(pall_sb[:], pall[:])
                ones8 = small.tile([8, 1], f32, tag="ones8")
                nc.vector.memset(ones8[:], 1.0)
                osb = prep.tile([1, NGROUP * B], f32, tag="osb")
                for g in range(NGROUP):
                    ps2t = ps2.tile([1, B], f32, tag="psg2")
                    nc.tensor.matmul(ps2t[:], ones8[:],
                                     pall_sb[:, g * B:(g + 1) * B],
                                     start=True, stop=True)
                    nc.scalar.mul(osb[:, g * B:(g + 1) * B], ps2t[:], 1.0 / TAU)
            # consume the warm-up collective's (all-zero) output
            wsb2 = small.tile([1, 16], f32, tag="wsb2")
            nc.sync.dma_start(wsb2[:], warm[0:1, :])
            nc.vector.tensor_add(osb[:, :16], osb[:, :16], wsb2[:])
            nc.sync.dma_start(out_d[:], osb[:])

    nc.compile()
    return nc


def _wrap_idx(ii):
    w = ii.astype(np.int16).reshape(-1, 16).T
    return np.ascontiguousarray(np.tile(w, (8, 1)))


def _src_unit(i):
    """Row unit of layer-1/2 neuron i in the AllGathered [SH*128, NJ12*B]
    layout: shard s = i//2000, local t = i - 2000s, p = t%128, j = t//128;
    row = s*128+p, unit = row*NJ12 + j."""
    s = i // REAL12
    t = i - s * REAL12
    p = t % P
    j = t // P
    return (s * P + p) * NJ12 + j


def _pack_w(w_eff, nj):
    # local slot t = j*128 + p  ->  packed[p, j*16+g]
    return np.ascontiguousarray(
        w_eff.reshape(nj, P, 16).transpose(1, 0, 2).reshape(P, nj * 16)
    )


def _host_pack(inputs):
    x = np.asarray(inputs["x"], dtype=np.float32)
    w1 = np.asarray(inputs["w1"], dtype=np.float32)
    w2 = np.asarray(inputs["w2"], dtype=np.float32)
    w3 = np.asarray(inputs["w3"], dtype=np.float32)
    i1a = np.asarray(inputs["idx1a"]).astype(np.int64)
    i1b = np.asarray(inputs["idx1b"]).astype(np.int64)
    i2a = np.asarray(inputs["idx2a"]).astype(np.int64)
    i2b = np.asarray(inputs["idx2b"]).astype(np.int64)
    i3a = np.asarray(inputs["idx3a"]).astype(np.int64)
    i3b = np.asarray(inputs["idx3b"]).astype(np.int64)

    import ml_dtypes

    pad_row = np.full(16, -20.0, dtype=np.float32)
    pad_row[0] = 20.0  # softmax -> ~one-hot FALSE gate -> h = 0

    xTf = np.ascontiguousarray(x.T)  # [193, 512]

    per_shard = []
    for s in range(SH):
        m = {}
        sel = slice(s * REAL12, (s + 1) * REAL12)

        # layer 1: one-hot matrices + packed weights
        w1_eff = np.concatenate(
            [w1[sel], np.tile(pad_row, (NS12 - REAL12, 1))], axis=0
        )
        m["w1p"] = _pack_w(w1_eff, NJ12)
        oh = np.zeros((IN, 2 * NS12), dtype=ml_dtypes.bfloat16)
        cols = np.arange(REAL12)
        oh[i1a[sel], cols] = 1.0
        oh[i1b[sel], NS12 + cols] = 1.0
        m["oh0"] = np.ascontiguousarray(oh[0:P])
        m["oh1"] = np.ascontiguousarray(oh[P:IN])

        # layer 2
        w2_eff = np.concatenate(
            [w2[sel], np.tile(pad_row, (NS12 - REAL12, 1))], axis=0
        )
        m["w2p"] = _pack_w(w2_eff, NJ12)
        ia = np.zeros(NS12, dtype=np.int64)
        ib = np.zeros(NS12, dtype=np.int64)
        ia[:REAL12] = _src_unit(i2a[sel])
        ib[:REAL12] = _src_unit(i2b[sel])
        m["i2"] = _wrap_idx(np.concatenate([ia, ib]))

        # layer 3: group g's 5333 real neurons split per CNT3; within (s, g):
        # local j in [6g, 6g+6), rank m = (j-6g)*128 + p
        u = np.arange(NS3)
        jj = u // P
        pp = u % P
        gg = jj // JPG
        mm = (jj - gg * JPG) * P + pp
        real = mm < CNT3[s]
        rid = gg * SPG + OFF3[s] + np.minimum(mm, CNT3[s] - 1)
        w3_eff = w3[rid].copy()
        w3_eff[~real] = pad_row
        m["w3p"] = _pack_w(w3_eff, NJ3)
        i3a_eff = np.where(real, _src_unit(i3a[rid]), 0)
        i3b_eff = np.where(real, _src_unit(i3b[rid]), 0)
        m["i3"] = _wrap_idx(np.concatenate([i3a_eff, i3b_eff]))

        m["xT"] = xTf
        per_shard.append(m)
    return per_shard


LAST_RESULTS = None


def kernel(**inputs):
    global LAST_RESULTS
    from concourse.bass_utils import run_bass_kernel_spmd

    if "nc" not in _CACHE:
        _CACHE["nc"] = _build_nc()
    nc = _CACHE["nc"]

    in_maps = _host_pack(inputs)
    trace = bool(int(os.environ.get("KERNEL_TRACE", "0")))
    res = run_bass_kernel_spmd(
        nc, in_maps, core_ids=list(range(N_CORES)), trace=trace
    )
    LAST_RESULTS = res

    rc = res.results[0]["out"].reshape(NGROUP, B)
    return np.ascontiguousarray(rc.T.astype(np.float32))


# revision 15
# speedup vs baseline: 1.7112x; 1.2815x over previous
"""Trainium2 Bass kernel for a 3-layer difflogic network (nn_Net_48610439856713).

Math: each layer o computes softmax(w[o])·ops16(a, b) with a = h[:, ia[o]],
b = h[:, ib[o]].  The 16 relaxed logic gates are all affine in {1, a, b, ab},
so the layer reduces to  h' = C0 + C1·a + C2·b + C3·a·b  with 4 per-neuron
coefficients derived on-device from softmax(w).

v3 design (vs the 2x4 baseline):
  - 8-way neuron sharding, full batch B=512 per core.  Layer exchange is ONE
    8-rank AllGather per layer (RDH algorithm, ~66us for 16MB out), which
    beats any 4-rank collective shape on this chip.
  - Layer 1 is computed as one-hot matmuls on the (otherwise idle) TensorE:
    a/b operand rows land in PSUM, removing layer-1's dma_gather descriptor
    generation (the Q7 SWDGE desc-gen at ~8ns/index is a main bottleneck).
  - Layer 2/3 gathers use prepare_only=True dma_gather: Q7 descriptor
    generation runs EARLY (overlapped with layer-1 compute / AllGather-1),
    and trigger_dma fires the prepared DMA once the AllGather has landed.
    A probe DMA reading the AG output + a WAW dep via signals_writable
    pins each trigger after its collective completes (the rust-side dep
    deferral alone gates only on the collective *doorbell*).
  - The per-neuron affine combine runs as 6 whole-layer DVE ops using
    stride-0 broadcast APs for the coefficient operands (instead of 3 small
    scalar_tensor_tensor ops per 128-slot column).

Host-side bookkeeping is integer/layout only: slot permutations, index
relabeling through the packed layout, int16 index wrapping, weight-row
packing, one-hot matrix construction.  All float arithmetic (softmax,
combine, sums) runs on device.
"""

import os
import numpy as np

P = 128
B = 512                  # full batch on every core
IN = 193
NGROUP = 3
TAU = 100.0
N_CORES = 8
SH = 8                   # neuron shards
BG = 1
BC = B                   # batch per core

NJ12 = 16                # j-columns per shard, layers 1/2
REAL12 = 2000            # real neurons per shard, layers 1/2
NS12 = NJ12 * P          # 2048 slots per shard

NJ3 = 18                 # layer 3: 3 groups x 6 j-cols
JPG = 6
NS3 = NJ3 * P            # 2304 slots
SPG = 15999 // NGROUP    # 5333 real neurons per group
CNT3 = [667, 667, 667, 667, 667, 666, 666, 666]   # per-shard split of 5333
OFF3 = np.concatenate([[0], np.cumsum(CNT3)[:-1]])

HD = os.environ.get("KERNEL_HDT", "f8")   # exchanged-activation dtype
SP = bool(int(os.environ.get("KERNEL_SP", "0")))  # single_packet gathers

_CACHE = {}


def _build_nc():
    import concourse.bacc as bacc
    import concourse.tile as tile
    import concourse.mybir as mybir

    f32 = mybir.dt.float32
    bf16 = mybir.dt.bfloat16
    i16 = mybir.dt.int16
    hdt = mybir.dt.float8e4 if HD == "f8" else bf16
    Alu = mybir.AluOpType
    Act = mybir.ActivationFunctionType
    Ax = mybir.AxisListType

    nc = bacc.Bacc("TRN2", target_bir_lowering=False, debug=False,
                   num_devices=N_CORES)

    G8 = [[0, 1, 2, 3, 4, 5, 6, 7]]

    # ---- I/O ----
    xT = nc.dram_tensor("xT", [IN, B], f32, kind="ExternalInput")
    oh0 = nc.dram_tensor("oh0", [P, 2 * NS12], bf16, kind="ExternalInput")
    oh1 = nc.dram_tensor("oh1", [IN - P, 2 * NS12], bf16, kind="ExternalInput")
    wps = [
        nc.dram_tensor("w1p", [P, NJ12 * 16], f32, kind="ExternalInput"),
        nc.dram_tensor("w2p", [P, NJ12 * 16], f32, kind="ExternalInput"),
        nc.dram_tensor("w3p", [P, NJ3 * 16], f32, kind="ExternalInput"),
    ]
    i2d = nc.dram_tensor("i2", [P, 2 * NS12 // 16], i16, kind="ExternalInput")
    i3d = nc.dram_tensor("i3", [P, 2 * NS3 // 16], i16, kind="ExternalInput")
    out_d = nc.dram_tensor("out", [1, NGROUP * B], f32, kind="ExternalOutput")

    # collective buffers
    cin = [nc.dram_tensor(f"cin{l}", [P, NJ12 * B], hdt, kind="Internal")
           for l in (1, 2)]
    gs_ = [nc.dram_tensor(f"g{l}", [SH * P, NJ12 * B], hdt, kind="Internal",
                          addr_space="Shared")
           for l in (1, 2)]
    win = nc.dram_tensor("win", [1, 16], f32, kind="Internal")
    warm = nc.dram_tensor("warm", [8, 16], f32, kind="Internal",
                          addr_space="Shared")
    pin = nc.dram_tensor("pin", [1, NGROUP * B], f32, kind="Internal")
    pall = nc.dram_tensor("pall", [8, NGROUP * B], f32, kind="Internal",
                          addr_space="Shared")

    def coeffs(pool, wp, nj, li):
        """softmax(w) -> affine coefficients C0..C3, each [P, nj] f32."""
        tg = lambda name: f"{name}_{li}"
        wt = pool.tile([P, nj * 16], f32, tag=tg("wt"))
        nc.sync.dma_start(wt[:], wp[:])
        e = pool.tile([P, nj * 16], f32, tag=tg("e"))
        nc.scalar.activation(e[:], wt[:], Act.Exp)
        e3 = e[:].rearrange("p (j g) -> p j g", g=16)
        e4 = e[:].rearrange("p (j h q) -> p j h q", h=4, q=4)

        ssum = pool.tile([P, nj], f32, tag=tg("ssum"))
        nc.vector.reduce_sum(ssum[:], e3, axis=Ax.X)
        r = pool.tile([P, nj], f32, tag=tg("r"))
        nc.vector.reciprocal(r[:], ssum[:])

        c0 = pool.tile([P, nj], f32, tag=tg("c0"))
        c1 = pool.tile([P, nj], f32, tag=tg("c1"))
        c2 = pool.tile([P, nj], f32, tag=tg("c2"))
        c3 = pool.tile([P, nj], f32, tag=tg("c3"))

        nc.vector.reduce_sum(c0[:], e4[:, :, 2:4, :], axis=Ax.XY)
        t1 = pool.tile([P, nj], f32, tag=tg("t1"))
        t2 = pool.tile([P, nj], f32, tag=tg("t2"))
        nc.vector.reduce_sum(t1[:], e4[:, :, 0:2, 2:4], axis=Ax.XY)
        nc.vector.reduce_sum(t2[:], e4[:, :, 2:4, 0:2], axis=Ax.XY)
        nc.vector.tensor_sub(c1[:], t1[:], t2[:])
        nc.vector.reduce_sum(t1[:], e4[:, :, 1, :], axis=Ax.X)
        nc.vector.reduce_sum(t2[:], e4[:, :, 2, :], axis=Ax.X)
        nc.vector.tensor_sub(c2[:], t1[:], t2[:])
        f = pool.tile([P, nj, 7], f32, tag=tg("f"))
        nc.vector.tensor_sub(f[:], e3[:, :, 1:8], e3[:, :, 14:7:-1])
        u1 = pool.tile([P, nj], f32, tag=tg("u1"))
        u2 = pool.tile([P, nj], f32, tag=tg("u2"))
        nc.vector.tensor_sub(u1[:], f[:, :, 0], f[:, :, 1])
        nc.vector.tensor_add(u2[:], f[:, :, 3], f[:, :, 6])
        nc.vector.tensor_sub(u1[:], u1[:], u2[:])
        nc.vector.scalar_tensor_tensor(
            c3[:], f[:, :, 5], -2.0, u1[:], op0=Alu.mult, op1=Alu.add
        )
        for ck in (c0, c1, c2, c3):
            nc.vector.tensor_mul(ck[:], ck[:], r[:])
        return c0, c1, c2, c3

    def combine(a3, b3, cs, hout, vsl, usl, nj, bc, dsA=None, dsB=None,
                nhalf=2):
        """hout = C0 + C1 a + C2 b + C3 ab over [P, nj, bc].

        The DVE runs at ~1 column/cycle per pass regardless of dtype, so
        passes are minimized: ScalarE computes the two per-j affine maps
        v = C3 a + C2 and u = C1 a + C0 (activation with per-partition
        scale+bias, reading a directly), and the DVE needs only
        t = v*b and h = t + u, done per-half so it can start before the
        full v/u chains finish.

        dsA/dsB: DMA-completion semaphores of the (split) prepared gathers
        for the a/b operand halves; attached as waits on the ops that read
        them (a bare engine wait_ge has no data deps, so the Tile scheduler
        is free to hoist it into a deadlock).  v/u only need `a`, so they
        start as soon as the a-half of the gather has landed."""
        c0, c1, c2, c3 = cs
        for j in range(nj):
            iv = nc.scalar.activation(vsl[:, j], a3[:, j], Act.Identity,
                                      bias=c2[:, j:j + 1],
                                      scale=c3[:, j:j + 1])
            if dsA is not None:
                iv.wait_op(dsA, 16, "sem-ge")
            iu = nc.scalar.activation(usl[:, j], a3[:, j], Act.Identity,
                                      bias=c0[:, j:j + 1],
                                      scale=c1[:, j:j + 1])
            if dsA is not None:
                iu.wait_op(dsA, 16, "sem-ge")
        step = (nj + nhalf - 1) // nhalf
        for h0 in range(0, nj, step):
            h1 = min(h0 + step, nj)
            it = nc.vector.tensor_mul(vsl[:, h0:h1], vsl[:, h0:h1],
                                      b3[:, h0:h1])
            if dsB is not None:
                it.wait_op(dsB, 16, "sem-ge")
            nc.vector.tensor_add(hout[:, h0:h1], vsl[:, h0:h1],
                                 usl[:, h0:h1])

    with tile.TileContext(nc) as tc:
        with (
            tc.tile_pool(name="big", bufs=1) as big,
            tc.tile_pool(name="prep", bufs=1) as prep,
            tc.tile_pool(name="small", bufs=2) as small,
        ):
            ds2a = nc.alloc_semaphore("ds2a")
            ds2b = nc.alloc_semaphore("ds2b")
            ds3a = nc.alloc_semaphore("ds3a")
            ds3b = nc.alloc_semaphore("ds3b")

            # shared combine scratch (sliced per layer)
            ct = big.tile([P, NJ3, B], bf16, tag="ct")
            cu = big.tile([P, NJ3, B], bf16, tag="cu")

            # ---- warm-up collective (absorbs first-collective barrier) ----
            wsb = small.tile([1, 16], f32, tag="wsb")
            nc.vector.memset(wsb[:], 0.0)
            nc.sync.dma_start(win[:], wsb[:])
            nc.gpsimd.collective_compute(
                "AllGather", Alu.bypass, replica_groups=G8,
                ins=[win[:]], outs=[warm[:]],
            )

            # ---- L2 gather prep (desc-gen runs now; DMA fires at trigger) ----
            i2t = small.tile([P, 2 * NS12 // 16], i16, tag="i2t")
            nc.sync.dma_start(i2t[:], i2d[:])
            ab2 = big.tile([P, 2 * NJ12, B], hdt, tag="ab2")
            g1rows = gs_[0][:].rearrange("r (j b) -> (r j) b", b=B)
            nc.gpsimd.dma_gather(
                ab2[:, 0:NJ12, :], g1rows, i2t[:, 0:NS12 // 16], NS12, NS12,
                B, prepare_only=True, sem=ds2a, single_packet=SP,
            )
            nc.gpsimd.dma_gather(
                ab2[:, NJ12:2 * NJ12, :], g1rows, i2t[:, NS12 // 16:], NS12,
                NS12, B, prepare_only=True, sem=ds2b, single_packet=SP,
            )

            # ---- layer 1: one-hot matmuls + combine ----
            cs1 = coeffs(prep, wps[0], NJ12, 1)
            h1 = big.tile([P, NJ12, B], hdt, tag="h")

            with tc.tile_pool(name="l1", bufs=1) as l1p:
                xs0 = l1p.tile([P, B], f32, tag="xs0")
                xs1 = l1p.tile([IN - P, B], f32, tag="xs1")
                nc.sync.dma_start(xs0[:], xT[0:P, :])
                nc.sync.dma_start(xs1[:], xT[P:IN, :])
                xb0 = l1p.tile([P, B], bf16, tag="xb0")
                xb1 = l1p.tile([IN - P, B], bf16, tag="xb1")
                nc.scalar.copy(xb0[:], xs0[:])
                nc.scalar.copy(xb1[:], xs1[:])
                oh0t = l1p.tile([P, 2 * NS12], bf16, tag="oh0t")
                oh1t = l1p.tile([IN - P, 2 * NS12], bf16, tag="oh1t")
                nc.sync.dma_start(oh0t[:], oh0[:])
                nc.sync.dma_start(oh1t[:], oh1[:])

                # a/b operand rows land in PSUM chunks; ScalarE computes
                # v = C3 a + C2 and u = C1 a + C0 straight from the a-PSUM
                # (fusing evacuation and affine map in one activation pass),
                # then the DVE finishes h = v*b + u per chunk.
                c0, c1, c2, c3 = cs1
                JC = 2  # j-columns per PSUM chunk (2 banks each for a and b)
                with tc.tile_pool(name="ps1", bufs=2, space="PSUM") as ps1:
                    for jc in range(0, NJ12, JC):
                        pa = ps1.tile([P, JC, B], f32, tag="pa")
                        pb = ps1.tile([P, JC, B], f32, tag="pb")
                        for jj in range(JC):
                            j = jc + jj
                            ca = j * P
                            cb = NS12 + j * P
                            nc.tensor.matmul(pa[:, jj], oh0t[:, ca:ca + P],
                                             xb0[:], start=True, stop=False)
                            nc.tensor.matmul(pa[:, jj], oh1t[:, ca:ca + P],
                                             xb1[:], start=False, stop=True)
                            nc.tensor.matmul(pb[:, jj], oh0t[:, cb:cb + P],
                                             xb0[:], start=True, stop=False)
                            nc.tensor.matmul(pb[:, jj], oh1t[:, cb:cb + P],
                                             xb1[:], start=False, stop=True)
                        for jj in range(JC):
                            j = jc + jj
                            nc.scalar.activation(ct[:, j], pa[:, jj],
                                                 Act.Identity,
                                                 bias=c2[:, j:j + 1],
                                                 scale=c3[:, j:j + 1])
                            nc.scalar.activation(cu[:, j], pa[:, jj],
                                                 Act.Identity,
                                                 bias=c0[:, j:j + 1],
                                                 scale=c1[:, j:j + 1])
                        nc.vector.tensor_mul(ct[:, jc:jc + JC],
                                             ct[:, jc:jc + JC], pb[:])
                        nc.vector.tensor_add(h1[:, jc:jc + JC, :],
                                             ct[:, jc:jc + JC],
                                             cu[:, jc:jc + JC])

            nc.sync.dma_start(cin[0][:], h1[:].rearrange("p j b -> p (j b)"))
            nc.gpsimd.collective_compute(
                "AllGather", Alu.bypass, replica_groups=G8,
                ins=[cin[0][:]], outs=[gs_[0][:]],
            )

            # ---- trigger L2 gather once g1 has landed ----
            pr1 = small.tile([1, 64], hdt, tag="pr1")
            nc.sync.dma_start(pr1[:], gs_[0][0:1, 0:64])
            nc.gpsimd.trigger_dma(count=None, signals_writable=[pr1[:], ab2[:]])

            # ---- L3 gather prep (desc-gen after trigger2 in program order) ----
            i3t = small.tile([P, 2 * NS3 // 16], i16, tag="i3t")
            nc.sync.dma_start(i3t[:], i3d[:])
            ab3 = big.tile([P, 2 * NJ3, B], hdt, tag="ab3")
            g2rows = gs_[1][:].rearrange("r (j b) -> (r j) b", b=B)
            nc.gpsimd.dma_gather(
                ab3[:, 0:NJ3, :], g2rows, i3t[:, 0:NS3 // 16], NS3, NS3,
                B, prepare_only=True, sem=ds3a, single_packet=SP,
            )
            nc.gpsimd.dma_gather(
                ab3[:, NJ3:2 * NJ3, :], g2rows, i3t[:, NS3 // 16:], NS3,
                NS3, B, prepare_only=True, sem=ds3b, single_packet=SP,
            )

            # ---- layer 2 ----
            cs2 = coeffs(prep, wps[1], NJ12, 2)
            h2 = big.tile([P, NJ12, B], hdt, tag="h")
            combine(ab2[:, 0:NJ12, :], ab2[:, NJ12:2 * NJ12, :], cs2, h2[:],
                    ct[:, 0:NJ12, :], cu[:, 0:NJ12, :], NJ12, B,
                    dsA=ds2a, dsB=ds2b)
            nc.sync.dma_start(cin[1][:], h2[:].rearrange("p j b -> p (j b)"))
            nc.gpsimd.collective_compute(
                "AllGather", Alu.bypass, replica_groups=G8,
                ins=[cin[1][:]], outs=[gs_[1][:]],
            )

            # ---- trigger L3 gather ----
            pr2 = small.tile([1, 64], hdt, tag="pr2")
            nc.sync.dma_start(pr2[:], gs_[1][0:1, 0:64])
            nc.gpsimd.trigger_dma(count=None, signals_writable=[pr2[:], ab3[:]])

            # ---- layer 3 ----
            cs3 = coeffs(prep, wps[2], NJ3, 3)
            h3 = big.tile([P, NJ3, B], bf16, tag="h3")
            combine(ab3[:, 0:NJ3, :], ab3[:, NJ3:2 * NJ3, :], cs3, h3[:],
                    ct[:], cu[:], NJ3, B, dsA=ds3a, dsB=ds3b, nhalf=3)

            # ---- GroupSum ----
            gsum = prep.tile([P, NGROUP * B], f32, tag="gsum")
            for g in range(NGROUP):
                sl = h3[:, g * JPG:(g + 1) * JPG, :].rearrange("p j b -> p b j")
                nc.vector.reduce_sum(gsum[:, g * B:(g + 1) * B], sl, axis=Ax.X)
            ones = small.tile([P, 1], f32, tag="ones")
            nc.vector.memset(ones[:], 1.0)
            psc = prep.tile([1, NGROUP * B], f32, tag="psc")
            with tc.tile_pool(name="ps2", bufs=2, space="PSUM") as ps2:
                for g in range(NGROUP):
                    ps = ps2.tile([1, B], f32, tag="psg")
                    nc.tensor.matmul(ps[:], ones[:], gsum[:, g * B:(g + 1) * B],
                                     start=True, stop=True)
                    nc.scalar.copy(psc[:, g * B:(g + 1) * B], ps[:])
                nc.sync.dma_start(pin[:], psc[:])
                nc.gpsimd.collective_compute(
                    "AllGather", Alu.bypass, replica_groups=G8,
                    ins=[pin[:]], outs=[pall[:]],
                )
                pall_sb = prep.tile([8, NGROUP * B], f32, tag="pall_sb")
                nc.sync.dma_start(pall_sb[:], pall[:])
                ones8 = small.tile([8, 1], f32, tag="ones8")
                nc.vector.memset(ones8[:], 1.0)
                osb = prep.tile([1, NGROUP * B], f32, tag="osb")
                for g in range(NGROUP):
                    ps2t = ps2.tile([1, B], f32, tag="psg2")
                    nc.tensor.matmul(ps2t[:], ones8[:],
                                     pall_sb[:, g * B:(g + 1) * B],
                                     start=True, stop=True)
                    nc.scalar.mul(osb[:, g * B:(g + 1) * B], ps2t[:], 1.0 / TAU)
            # consume the warm-up collective's (all-zero) output
            wsb2 = small.tile([1, 16], f32, tag="wsb2")
            nc.sync.dma_start(wsb2[:], warm[0:1, :])
            nc.vector.tensor_add(osb[:, :16], osb[:, :16], wsb2[:])
            nc.sync.dma_start(out_d[:], osb[:])

    nc.compile()
    return nc


def _wrap_idx(ii):
    w = ii.astype(np.int16).reshape(-1, 16).T
    return np.ascontiguousarray(np.tile(w, (8, 1)))


def _src_unit(i):
    """Row unit of layer-1/2 neuron i in the AllGathered [SH*128, NJ12*B]
    layout: shard s = i//2000, local t = i - 2000s, p = t%128, j = t//128;
    row = s*128+p, unit = row*NJ12 + j."""
    s = i // REAL12
    t = i - s * REAL12
    p = t % P
    j = t // P
    return (s * P + p) * NJ12 + j


def _pack_w(w_eff, nj):
    # local slot t = j*128 + p  ->  packed[p, j*16+g]
    return np.ascontiguousarray(
        w_eff.reshape(nj, P, 16).transpose(1, 0, 2).reshape(P, nj * 16)
    )


def _host_pack(inputs):
    x = np.asarray(inputs["x"], dtype=np.float32)
    w1 = np.asarray(inputs["w1"], dtype=np.float32)
    w2 = np.asarray(inputs["w2"], dtype=np.float32)
    w3 = np.asarray(inputs["w3"], dtype=np.float32)
    i1a = np.asarray(inputs["idx1a"]).astype(np.int64)
    i1b = np.asarray(inputs["idx1b"]).astype(np.int64)
    i2a = np.asarray(inputs["idx2a"]).astype(np.int64)
    i2b = np.asarray(inputs["idx2b"]).astype(np.int64)
    i3a = np.asarray(inputs["idx3a"]).astype(np.int64)
    i3b = np.asarray(inputs["idx3b"]).astype(np.int64)

    import ml_dtypes

    pad_row = np.full(16, -20.0, dtype=np.float32)
    pad_row[0] = 20.0  # softmax -> ~one-hot FALSE gate -> h = 0

    xTf = np.ascontiguousarray(x.T)  # [193, 512]

    per_shard = []
    for s in range(SH):
        m = {}
        sel = slice(s * REAL12, (s + 1) * REAL12)

        # layer 1: one-hot matrices + packed weights
        w1_eff = np.concatenate(
            [w1[sel], np.tile(pad_row, (NS12 - REAL12, 1))], axis=0
        )
        m["w1p"] = _pack_w(w1_eff, NJ12)
        oh = np.zeros((IN, 2 * NS12), dtype=ml_dtypes.bfloat16)
        cols = np.arange(REAL12)
        oh[i1a[sel], cols] = 1.0
        oh[i1b[sel], NS12 + cols] = 1.0
        m["oh0"] = np.ascontiguousarray(oh[0:P])
        m["oh1"] = np.ascontiguousarray(oh[P:IN])

        # layer 2
        w2_eff = np.concatenate(
            [w2[sel], np.tile(pad_row, (NS12 - REAL12, 1))], axis=0
        )
        m["w2p"] = _pack_w(w2_eff, NJ12)
        ia = np.zeros(NS12, dtype=np.int64)
        ib = np.zeros(NS12, dtype=np.int64)
        ia[:REAL12] = _src_unit(i2a[sel])
        ib[:REAL12] = _src_unit(i2b[sel])
        m["i2"] = _wrap_idx(np.concatenate([ia, ib]))

        # layer 3: group g's 5333 real neurons split per CNT3; within (s, g):
        # local j in [6g, 6g+6), rank m = (j-6g)*128 + p
        u = np.arange(NS3)
        jj = u // P
        pp = u % P
        gg = jj // JPG
        mm = (jj - gg * JPG) * P + pp
        real = mm < CNT3[s]
        rid = gg * SPG + OFF3[s] + np.minimum(mm, CNT3[s] - 1)
        w3_eff = w3[rid].copy()
        w3_eff[~real] = pad_row
        m["w3p"] = _pack_w(w3_eff, NJ3)
        i3a_eff = np.where(real, _src_unit(i3a[rid]), 0)
        i3b_eff = np.where(real, _src_unit(i3b[rid]), 0)
        m["i3"] = _wrap_idx(np.concatenate([i3a_eff, i3b_eff]))

        m["xT"] = xTf
        per_shard.append(m)
    return per_shard


LAST_RESULTS = None


def kernel(**inputs):
    global LAST_RESULTS
    from concourse.bass_utils import run_bass_kernel_spmd

    if "nc" not in _CACHE:
        _CACHE["nc"] = _build_nc()
    nc = _CACHE["nc"]

    in_maps = _host_pack(inputs)
    trace = bool(int(os.environ.get("KERNEL_TRACE", "0")))
    res = run_bass_kernel_spmd(
        nc, in_maps, core_ids=list(range(N_CORES)), trace=trace
    )
    LAST_RESULTS = res

    rc = res.results[0]["out"].reshape(NGROUP, B)
    return np.ascontiguousarray(rc.T.astype(np.float32))
